# revision 25
# baseline (speedup 1.0000x reference)
"""Trainium2 Bass kernel for InvariantMessagePassingTP.

out[n, lm, c] = sum_{e: recv[e]=n} edge_attrs[e,lm] * tp_weights[e,l(lm),c]
                * node_feats[recv[e], c]

Key identity: within a segment recv[e]=n, node_feats factors OUT of the sum:
  out[n] = node_feats[n] (broadcast over lm) * S[n],
  S[n,lm,c] = sum_{e->n} edge_attrs[e,lm] * tp_weights[e,l(lm),c].
The device computes only S; the host applies the F multiply (free) while
gathering. This removes the per-edge F stream and the U=W*F multiply.

Strategy (8 NeuronCores, SPMD, no collectives):
  receiver_list is sorted -> each core owns a contiguous node range (3125
  nodes) and its edges. Host bin-packs nodes into tiles: <=8 nodes and
  <=128 edges per tile (folded pairing, ~99% fill). Edges sit on SBUF
  partitions.

  Per tile:
    At[e, lm*8+k] = A2[e,lm-pair] * S8[e,k]   (DVE TT bf16 2x, batched
                    per 8-tile PSUM batch; S8 = one-hot of slot k)
    mmA: PSUM[c', lm*8+k (0:32)]  += W[:,0:128]^T  @ At[:, 0:32]
    mmB: PSUM[c', lm*8+k (32:128)] += W[:,128:256]^T @ At[:, 32:128]
  (W half as the 128-col stationary -> fast weight load; 128 moving
  cols per tile total.) Useful rectangles per l are extracted
  (ACT: l3,l2; DVE: l1,l0) to bf16 staging and DMA'd as per-l blocks.
  Host scatters slots -> S[node, lm, c] (each node owned by exactly one
  slot), multiplies by node_feats, and emits [nnodes, 16, 64] fp32.
"""

import sys

sys.path.insert(0, "/opt/trn_rl_repo")

import numpy as np
import ml_dtypes

import concourse.bass as bass
import concourse.bacc as bacc
import concourse.tile as tile
from concourse import mybir
from concourse.bass_utils import run_bass_kernel_spmd

NPBF = ml_dtypes.bfloat16
BF16 = mybir.dt.bfloat16
F32 = mybir.dt.float32

NNODES = 25000
NEDGES = 400000
NCHAN = 64
N_CORES = 8
NPC = NNODES // N_CORES        # nodes per core
TB = 273                       # bf16 elems per tile per partition (W256+A16+loc)
OB = 80                        # out cols per tile per partition (high half)
CHUNK = 32                     # tiles per input DMA chunk
PSB = 8                        # tiles per PSUM batch

M_L = [1, 3, 5, 7]             # lm multiplicity per l
LM0 = [0, 1, 4, 9]             # first lm of each l

_PROGRAM_CACHE = {}


def _fold_pack(degs):
    """Bin nodes (<=8 per bin, <=128 edges per bin) by folded pairing:
    sort by degree, pair k-th smallest with k-th largest, 3 levels ->
    8-node bins with near-equal edge sums; overfull bins shed smallest
    nodes which are then best-fit into remaining capacity."""
    items = [([i], int(degs[i])) for i in np.argsort(degs, kind="stable")]
    for _ in range(3):
        if len(items) % 2:
            items.append(([], 0))
        m = len(items)
        merged = [
            (items[i][0] + items[m - 1 - i][0], items[i][1] + items[m - 1 - i][1])
            for i in range(m // 2)
        ]
        merged.sort(key=lambda x: x[1])
        items = merged
    bins, loads, spill = [], [], []
    for nodes, s in items:
        nodes = sorted(nodes, key=lambda x: -degs[x])
        while s > 128 and nodes:
            v = nodes.pop()
            s -= int(degs[v])
            spill.append(v)
        if nodes:
            bins.append(nodes)
            loads.append(int(sum(int(degs[v]) for v in nodes)))
    spill.sort(key=lambda x: -degs[x])
    for v in spill:
        dv = int(degs[v])
        best, bestcap = -1, 1000
        for b in range(len(bins)):
            cap = 128 - loads[b]
            if cap >= dv and len(bins[b]) < 8 and cap < bestcap:
                best, bestcap = b, cap
        if best >= 0:
            bins[best].append(v)
            loads[best] += dv
        else:
            bins.append([v])
            loads.append(dv)
    return bins


def _build_schedule(receiver_list):
    recv = np.asarray(receiver_list).astype(np.int64)
    deg = np.bincount(recv, minlength=NNODES)
    node_e0 = np.searchsorted(recv, np.arange(NNODES + 1))

    # balance contiguous node ranges so every core packs into <= T* tiles
    def t_of(b0, b1):
        return len(_fold_pack(deg[b0:b1])) if b1 > b0 else 0

    bounds = None
    for t_star in range(393, 441):
        bs, ok = [0], True
        for c in range(N_CORES):
            b0 = bs[-1]
            if c == N_CORES - 1:
                b1 = NNODES
                ok = t_of(b0, b1) <= t_star
            else:
                lo, hi = b0, min(NNODES, b0 + 8 * t_star)
                while lo < hi:
                    mid = (lo + hi + 1) // 2
                    if t_of(b0, mid) <= t_star:
                        lo = mid
                    else:
                        hi = mid - 1
                b1 = lo
            bs.append(b1)
            if not ok:
                break
        if ok and bs[-1] == NNODES:
            bounds = bs
            break
    if bounds is None:
        bounds = [i * NPC for i in range(N_CORES + 1)]
    per_core = [_fold_pack(deg[bounds[c]:bounds[c + 1]])
                for c in range(N_CORES)]
    t_u = max(len(b) for b in per_core)
    t_u = -(-t_u // PSB) * PSB
    return deg, node_e0, per_core, t_u, bounds


def _pack_core(bins, t_u, deg, node_e0, w_bf, a2_bf, b0, n_c):
    """Build the [128, T*TB] input buffer and the node map for one core."""
    T = t_u
    # per-slot node lists -> per-edge (tile, slot, edge-idx) arrays
    tile_id, k_id, nodes = [], [], []
    node_map = np.full((T, 8), n_c, np.int32)  # local node id, n_c = dummy
    for t, b in enumerate(bins):
        for k, v in enumerate(b):
            tile_id.append(t)
            k_id.append(k)
            nodes.append(v)
            node_map[t, k] = v
    tile_id = np.array(tile_id, np.int64)
    k_id = np.array(k_id, np.int64)
    nodes = np.array(nodes, np.int64)
    gnodes = nodes + b0
    lens = deg[gnodes]
    starts = node_e0[gnodes]
    total = int(lens.sum())
    # concatenated edge indices per slot order
    step = np.ones(total, np.int64)
    ends = np.cumsum(lens)
    step[0] = starts[0]
    step[ends[:-1]] = starts[1:] - (starts[:-1] + lens[:-1] - 1)
    e_idx = np.cumsum(step)
    e_tile = np.repeat(tile_id, lens)
    e_k = np.repeat(k_id, lens)
    # position within tile (edges are emitted grouped by tile in slot order)
    tile_lens = np.bincount(e_tile, minlength=T)
    tile_base = np.concatenate(([0], np.cumsum(tile_lens)[:-1]))
    pos = np.arange(total) - np.repeat(tile_base, tile_lens)

    E_idx = np.full((T, 128), len(w_bf) - 1, np.int64)  # pad -> zero row
    loc = np.full((T, 128), 8, np.int64)                # pad -> zero one-hot
    E_idx[e_tile, pos] = e_idx
    loc[e_tile, pos] = e_k

    X = np.zeros((128, T * TB), NPBF)
    n_chunks = -(-T // CHUNK)
    for ch in range(n_chunks):
        t0, t1 = ch * CHUNK, min((ch + 1) * CHUNK, T)
        ct = t1 - t0
        base = t0 * TB
        a_blk = a2_bf[E_idx[t0:t1]]                      # [ct,128,16]
        l_blk = loc[t0:t1][:, :, None].astype(NPBF)      # [ct,128,1] slot id
        as_blk = np.concatenate([a_blk, l_blk], axis=2)  # [ct,128,17]
        X[:, base:base + ct * 17] = (
            as_blk.transpose(1, 0, 2).reshape(128, ct * 17))
        w_blk = w_bf[E_idx[t0:t1]]                       # [ct,128,256]
        X[:, base + ct * 17:base + ct * TB] = (
            w_blk.transpose(1, 0, 2).reshape(128, ct * 256))
    return X, node_map


def _build_program(t_u):
    nc = bacc.Bacc("TRN2", target_bir_lowering=False, debug=False,
                   num_devices=N_CORES)
    T = t_u
    in_d = nc.dram_tensor("inp", [128, T * TB], BF16, kind="ExternalInput").ap()
    out_d = nc.dram_tensor("out", [128, T * OB], BF16,
                           kind="ExternalOutput").ap()

    n_chunks = -(-T // CHUNK)
    with tile.TileContext(nc) as tc:
        with tc.tile_pool(name="cst", bufs=1) as cst_pool, \
             tc.tile_pool(name="as_", bufs=4) as as_pool, \
             tc.tile_pool(name="w", bufs=4) as w_pool, \
             tc.tile_pool(name="s8", bufs=4) as s8_pool, \
             tc.tile_pool(name="at", bufs=4) as at_pool, \
             tc.tile_pool(name="st", bufs=3) as st_pool, \
             tc.tile_pool(name="ps", bufs=4, space="PSUM") as ps_pool:
            iota_t = cst_pool.tile([128, 8], BF16, tag="iota")
            nc.gpsimd.iota(iota_t, pattern=[[1, 8]], base=0,
                           channel_multiplier=0,
                           allow_small_or_imprecise_dtypes=True)
            for ch in range(n_chunks):
                t0, t1 = ch * CHUNK, min((ch + 1) * CHUNK, T)
                ct = t1 - t0
                base = t0 * TB
                as_t = as_pool.tile([128, ct * 17], BF16, tag="as_")
                nc.sync.dma_start(
                    out=as_t,
                    in_=bass.AP(tensor=in_d.tensor, offset=base,
                                ap=[[T * TB, 128], [1, ct * 17]]),
                )
                w_t = w_pool.tile([128, ct * 256], BF16, tag="w")
                nc.sync.dma_start(
                    out=w_t,
                    in_=bass.AP(tensor=in_d.tensor, offset=base + ct * 17,
                                ap=[[T * TB, 128], [1, ct * 256]]),
                )
                stage = st_pool.tile([128, ct * OB], BF16, tag="stage")
                st0 = stage[0:64, 0:ct * 8].rearrange(
                    "p (t k) -> p t k", k=8)
                st2 = stage[0:64, ct * 8:ct * 48].rearrange(
                    "p (t j) -> p t j", j=40)
                st1 = stage[64:128, 0:ct * 24].rearrange(
                    "p (t j) -> p t j", j=24)
                st3 = stage[64:128, ct * 24:ct * 80].rearrange(
                    "p (t j) -> p t j", j=56)
                for pb in range(ct // PSB):
                    p0 = pb * PSB
                    # S8[e, t, k] = (loc[e, t] == k)  (one-hot)
                    s8 = s8_pool.tile([128, PSB * 8], BF16, tag="s8")
                    nc.vector.tensor_tensor(
                        s8.rearrange("p (t k) -> p t k", t=PSB),
                        bass.AP(tensor=as_t.tensor,
                                offset=as_t.offset + p0 * 17 + 16,
                                ap=[as_t.ap[0], [17, PSB], [0, 8]]),
                        bass.AP(tensor=iota_t.tensor, offset=iota_t.offset,
                                ap=[iota_t.ap[0], [0, PSB], [1, 8]]),
                        mybir.AluOpType.is_equal,
                    )
                    at = at_pool.tile([128, PSB * 128], BF16, tag="at")
                    # At[e, t, lm*8 + k] = A[e, lm] * S8[e, k]
                    nc.vector.tensor_mul(
                        at.rearrange("p (t l k) -> p t l k", t=PSB, l=16),
                        bass.AP(tensor=as_t.tensor,
                                offset=as_t.offset + p0 * 17,
                                ap=[as_t.ap[0], [17, PSB], [1, 16],
                                    [0, 8]]),
                        bass.AP(tensor=s8.tensor, offset=s8.offset,
                                ap=[s8.ap[0], [8, PSB], [0, 16],
                                    [1, 8]]),
                    )
                    ps = ps_pool.tile([128, PSB, 128], F32, tag="ps")
                    for k in range(PSB):
                        t = p0 + k
                        nc.tensor.matmul(
                            ps[:, k, 0:32],
                            w_t[:, t * 256:t * 256 + 128],
                            at[:, k * 128:k * 128 + 32],
                            start=True, stop=True)
                        nc.tensor.matmul(
                            ps[:, k, 32:128],
                            w_t[:, t * 256 + 128:t * 256 + 256],
                            at[:, k * 128 + 32:k * 128 + 128],
                            start=True, stop=True)
                    # useful-rectangle extraction (fp32 PSUM -> bf16 stage)
                    nc.scalar.copy(st3[:, p0:p0 + PSB], ps[64:128, :, 72:128])
                    nc.scalar.copy(st2[:, p0:p0 + PSB], ps[0:64, :, 32:72])
                    nc.scalar.copy(st1[:, p0:p0 + PSB], ps[64:128, :, 8:32])
                    nc.vector.tensor_copy(st0[:, p0:p0 + PSB],
                                          ps[0:64, :, 0:8])
                # chunk-major output blocks on the Pool queue (keeps the SP
                # queue free for input prefetch): rows 0:64 = [l0 ct*8 |
                # l2 ct*40] @ t0*48, rows 64:128 = [l1 ct*24 | l3 ct*56]
                # @ t0*80
                nc.gpsimd.dma_start(
                    out=bass.AP(tensor=out_d.tensor, offset=t0 * 48,
                                ap=[[T * OB, 64], [1, ct * 48]]),
                    in_=stage[0:64, 0:ct * 48])
                nc.gpsimd.dma_start(
                    out=bass.AP(tensor=out_d.tensor,
                                offset=64 * T * OB + t0 * 80,
                                ap=[[T * OB, 64], [1, ct * 80]]),
                    in_=stage[64:128, 0:ct * 80])
    nc.compile()
    return nc


def kernel(node_feats, edge_attrs, tp_weights, receiver_list, nnodes,
           _trace=False):
    node_feats = np.asarray(node_feats)
    edge_attrs = np.asarray(edge_attrs)
    tp_weights = np.asarray(tp_weights)
    receiver_list = np.asarray(receiver_list)
    nnodes = int(nnodes)
    assert node_feats.shape == (NNODES, NCHAN) and nnodes == NNODES
    assert tp_weights.shape == (NEDGES, 4, NCHAN)

    deg, node_e0, per_core, t_u, bounds = _build_schedule(receiver_list)
    key = int(t_u)
    if key not in _PROGRAM_CACHE:
        _PROGRAM_CACHE[key] = _build_program(t_u)
    nc = _PROGRAM_CACHE[key]

    # padded-by-one edge tables (last row = zeros) for gather packing
    w_bf = np.zeros((NEDGES + 1, 256), NPBF)
    w_bf[:NEDGES] = np.asarray(tp_weights, np.float32).reshape(
        NEDGES, 256).astype(NPBF)
    a2_bf = np.zeros((NEDGES + 1, 16), NPBF)
    a2_bf[:NEDGES] = np.asarray(edge_attrs, np.float32).astype(NPBF)

    in_maps, node_maps = [], []
    for c in range(N_CORES):
        X, node_map = _pack_core(per_core[c], t_u, deg, node_e0,
                                 w_bf, a2_bf, bounds[c],
                                 bounds[c + 1] - bounds[c])
        in_maps.append({"inp": X})
        node_maps.append(node_map)

    res = run_bass_kernel_spmd(nc, in_maps, list(range(N_CORES)),
                               trace=_trace)

    T = t_u
    feats = np.asarray(node_feats, np.float32)
    out = np.empty((NNODES, 16, NCHAN), np.float32)
    for c in range(N_CORES):
        r = res.results[c]["out"].astype(np.float32)   # [128, T*80]
        lo = np.empty((64, T, 48), np.float32)
        hi = np.empty((64, T, 80), np.float32)
        for ch in range(-(-T // CHUNK)):
            t0, t1 = ch * CHUNK, min((ch + 1) * CHUNK, T)
            ct = t1 - t0
            lo_reg = r[0:64, t0 * 48:t0 * 48 + ct * 48]
            lo[:, t0:t1, 0:8] = lo_reg[:, 0:ct * 8].reshape(64, ct, 8)
            lo[:, t0:t1, 8:48] = lo_reg[:, ct * 8:].reshape(64, ct, 40)
            hi_reg = r[64:128, t0 * 80:t0 * 80 + ct * 80]
            hi[:, t0:t1, 0:24] = hi_reg[:, 0:ct * 24].reshape(64, ct, 24)
            hi[:, t0:t1, 24:80] = hi_reg[:, ct * 24:].reshape(64, ct, 56)
        b0, b1 = bounds[c], bounds[c + 1]
        n_c = b1 - b0
        S = np.empty((n_c + 1, 16, NCHAN), np.float32)
        idx = node_maps[c].ravel()                      # [T*8] local ids
        blocks = (
            (lo[:, :, 0:8].reshape(64, T, 1, 8), 0, 1),
            (hi[:, :, 0:24].reshape(64, T, 3, 8), 1, 3),
            (lo[:, :, 8:48].reshape(64, T, 5, 8), 4, 5),
            (hi[:, :, 24:80].reshape(64, T, 7, 8), 9, 7),
        )
        for blk, lm0, m in blocks:
            vals = blk.transpose(1, 3, 2, 0).reshape(T * 8, m, NCHAN)
            S[idx, lm0:lm0 + m] = vals
        out[b0:b1] = S[:n_c] * feats[b0:b1, None, :]
    if _trace:
        return out, res
    return out


# revision 26
# speedup vs baseline: 1.1669x; 1.1669x over previous
"""Trainium2 Bass kernel for InvariantMessagePassingTP.

out[n, lm, c] = sum_{e: recv[e]=n} edge_attrs[e,lm] * tp_weights[e,l(lm),c]
                * node_feats[recv[e], c]

Key identity: within a segment recv[e]=n, node_feats factors OUT of the sum:
  out[n] = node_feats[n] (broadcast over lm) * S[n],
  S[n,lm,c] = sum_{e->n} edge_attrs[e,lm] * tp_weights[e,l(lm),c].
The device computes only S; the host applies the F multiply (free) while
gathering. This removes the per-edge F stream and the U=W*F multiply.

Strategy (8 NeuronCores, SPMD, no collectives):
  receiver_list is sorted -> each core owns a contiguous node range (3125
  nodes) and its edges. Host bin-packs nodes into tiles: <=8 nodes and
  <=128 edges per tile (folded pairing, ~99% fill). Edges sit on SBUF
  partitions.

  Per tile:
    At[e, lm*8+k] = A2[e,lm-pair] * S8[e,k]   (DVE TT bf16 2x, batched
                    per 8-tile PSUM batch; S8 = one-hot of slot k)
    mmA: PSUM[c', lm*8+k (0:32)]  += W[:,0:128]^T  @ At[:, 0:32]
    mmB: PSUM[c', lm*8+k (32:128)] += W[:,128:256]^T @ At[:, 32:128]
  (W half as the 128-col stationary -> fast weight load; 128 moving
  cols per tile total.) Useful rectangles per l are extracted
  (ACT: l3,l2; DVE: l1,l0) to bf16 staging and DMA'd as per-l blocks.
  Host scatters slots -> S[node, lm, c] (each node owned by exactly one
  slot), multiplies by node_feats, and emits [nnodes, 16, 64] fp32.
"""

import sys

sys.path.insert(0, "/opt/trn_rl_repo")

import numpy as np
import ml_dtypes

import concourse.bass as bass
import concourse.bacc as bacc
import concourse.tile as tile
from concourse import mybir
from concourse.bass_utils import run_bass_kernel_spmd

NPBF = ml_dtypes.bfloat16
BF16 = mybir.dt.bfloat16
F32 = mybir.dt.float32

NNODES = 25000
NEDGES = 400000
NCHAN = 64
N_CORES = 8
NPC = NNODES // N_CORES        # nodes per core
TB = 274                       # bf16 elems/tile/partition (W256+A16+loc+pad)
OB = 80                        # out cols per tile per partition (high half)
CHUNK = 32                     # tiles per input DMA chunk
PSB = 8                        # tiles per PSUM batch

M_L = [1, 3, 5, 7]             # lm multiplicity per l
LM0 = [0, 1, 4, 9]             # first lm of each l

_PROGRAM_CACHE = {}


def _fold_pack(degs):
    """Bin nodes (<=8 per bin, <=128 edges per bin) by folded pairing:
    sort by degree, pair k-th smallest with k-th largest, 3 levels ->
    8-node bins with near-equal edge sums; overfull bins shed smallest
    nodes which are then best-fit into remaining capacity."""
    items = [([i], int(degs[i])) for i in np.argsort(degs, kind="stable")]
    for _ in range(3):
        if len(items) % 2:
            items.append(([], 0))
        m = len(items)
        merged = [
            (items[i][0] + items[m - 1 - i][0], items[i][1] + items[m - 1 - i][1])
            for i in range(m // 2)
        ]
        merged.sort(key=lambda x: x[1])
        items = merged
    bins, loads, spill = [], [], []
    for nodes, s in items:
        nodes = sorted(nodes, key=lambda x: -degs[x])
        while s > 128 and nodes:
            v = nodes.pop()
            s -= int(degs[v])
            spill.append(v)
        if nodes:
            bins.append(nodes)
            loads.append(int(sum(int(degs[v]) for v in nodes)))
    spill.sort(key=lambda x: -degs[x])
    for v in spill:
        dv = int(degs[v])
        best, bestcap = -1, 1000
        for b in range(len(bins)):
            cap = 128 - loads[b]
            if cap >= dv and len(bins[b]) < 8 and cap < bestcap:
                best, bestcap = b, cap
        if best >= 0:
            bins[best].append(v)
            loads[best] += dv
        else:
            bins.append([v])
            loads.append(dv)
    return bins


def _build_schedule(receiver_list):
    recv = np.asarray(receiver_list).astype(np.int64)
    deg = np.bincount(recv, minlength=NNODES)
    node_e0 = np.searchsorted(recv, np.arange(NNODES + 1))

    # balance contiguous node ranges so every core packs into <= T* tiles
    def t_of(b0, b1):
        return len(_fold_pack(deg[b0:b1])) if b1 > b0 else 0

    bounds = None
    for t_star in range(393, 441):
        bs, ok = [0], True
        for c in range(N_CORES):
            b0 = bs[-1]
            if c == N_CORES - 1:
                b1 = NNODES
                ok = t_of(b0, b1) <= t_star
            else:
                lo, hi = b0, min(NNODES, b0 + 8 * t_star)
                while lo < hi:
                    mid = (lo + hi + 1) // 2
                    if t_of(b0, mid) <= t_star:
                        lo = mid
                    else:
                        hi = mid - 1
                b1 = lo
            bs.append(b1)
            if not ok:
                break
        if ok and bs[-1] == NNODES:
            bounds = bs
            break
    if bounds is None:
        bounds = [i * NPC for i in range(N_CORES + 1)]
    per_core = [_fold_pack(deg[bounds[c]:bounds[c + 1]])
                for c in range(N_CORES)]
    t_u = max(len(b) for b in per_core)
    t_u = -(-t_u // 16) * 16   # x16: keeps DRAM row strides 64B-aligned
    return deg, node_e0, per_core, t_u, bounds


def _pack_core(bins, t_u, deg, node_e0, w_bf, a2_bf, b0, n_c):
    """Build the [128, T*TB] input buffer and the node map for one core."""
    T = t_u
    # per-slot node lists -> per-edge (tile, slot, edge-idx) arrays
    tile_id, k_id, nodes = [], [], []
    node_map = np.full((T, 8), n_c, np.int32)  # local node id, n_c = dummy
    for t, b in enumerate(bins):
        for k, v in enumerate(b):
            tile_id.append(t)
            k_id.append(k)
            nodes.append(v)
            node_map[t, k] = v
    tile_id = np.array(tile_id, np.int64)
    k_id = np.array(k_id, np.int64)
    nodes = np.array(nodes, np.int64)
    gnodes = nodes + b0
    lens = deg[gnodes]
    starts = node_e0[gnodes]
    total = int(lens.sum())
    # concatenated edge indices per slot order
    step = np.ones(total, np.int64)
    ends = np.cumsum(lens)
    step[0] = starts[0]
    step[ends[:-1]] = starts[1:] - (starts[:-1] + lens[:-1] - 1)
    e_idx = np.cumsum(step)
    e_tile = np.repeat(tile_id, lens)
    e_k = np.repeat(k_id, lens)
    # position within tile (edges are emitted grouped by tile in slot order)
    tile_lens = np.bincount(e_tile, minlength=T)
    tile_base = np.concatenate(([0], np.cumsum(tile_lens)[:-1]))
    pos = np.arange(total) - np.repeat(tile_base, tile_lens)

    E_idx = np.full((T, 128), len(w_bf) - 1, np.int64)  # pad -> zero row
    loc = np.full((T, 128), 8, np.int64)                # pad -> zero one-hot
    E_idx[e_tile, pos] = e_idx
    loc[e_tile, pos] = e_k

    X = np.zeros((128, T * TB), NPBF)
    n_chunks = -(-T // CHUNK)
    for ch in range(n_chunks):
        t0, t1 = ch * CHUNK, min((ch + 1) * CHUNK, T)
        ct = t1 - t0
        base = t0 * TB
        a_blk = a2_bf[E_idx[t0:t1]]                      # [ct,128,16]
        l_blk = loc[t0:t1][:, :, None].astype(NPBF)      # [ct,128,1] slot id
        pad = np.zeros((t1 - t0, 128, 1), NPBF)
        as_blk = np.concatenate([a_blk, l_blk, pad], axis=2)  # [ct,128,18]
        X[:, base:base + ct * 18] = (
            as_blk.transpose(1, 0, 2).reshape(128, ct * 18))
        w_blk = w_bf[E_idx[t0:t1]]                       # [ct,128,256]
        X[:, base + ct * 18:base + ct * TB] = (
            w_blk.transpose(1, 0, 2).reshape(128, ct * 256))
    return X, node_map


def _build_program(t_u):
    nc = bacc.Bacc("TRN2", target_bir_lowering=False, debug=False,
                   num_devices=N_CORES)
    T = t_u
    in_d = nc.dram_tensor("inp", [128, T * TB], BF16, kind="ExternalInput").ap()
    out_d = nc.dram_tensor("out", [128, T * OB], BF16,
                           kind="ExternalOutput").ap()

    n_chunks = -(-T // CHUNK)
    with tile.TileContext(nc) as tc:
        with tc.tile_pool(name="cst", bufs=1) as cst_pool, \
             tc.tile_pool(name="as_", bufs=4) as as_pool, \
             tc.tile_pool(name="w", bufs=4) as w_pool, \
             tc.tile_pool(name="s8", bufs=4) as s8_pool, \
             tc.tile_pool(name="at", bufs=4) as at_pool, \
             tc.tile_pool(name="st", bufs=3) as st_pool, \
             tc.tile_pool(name="ps", bufs=4, space="PSUM") as ps_pool:
            iota_t = cst_pool.tile([128, 8], BF16, tag="iota")
            nc.gpsimd.iota(iota_t, pattern=[[1, 8]], base=0,
                           channel_multiplier=0,
                           allow_small_or_imprecise_dtypes=True)
            for ch in range(n_chunks):
                t0, t1 = ch * CHUNK, min((ch + 1) * CHUNK, T)
                ct = t1 - t0
                base = t0 * TB
                as_t = as_pool.tile([128, ct * 18], BF16, tag="as_")
                nc.sync.dma_start(
                    out=as_t,
                    in_=bass.AP(tensor=in_d.tensor, offset=base,
                                ap=[[T * TB, 128], [1, ct * 18]]),
                )
                w_t = w_pool.tile([128, ct * 256], BF16, tag="w")
                nc.sync.dma_start(
                    out=w_t,
                    in_=bass.AP(tensor=in_d.tensor, offset=base + ct * 18,
                                ap=[[T * TB, 128], [1, ct * 256]]),
                )
                stage = st_pool.tile([128, ct * OB], BF16, tag="stage")
                st0 = stage[0:64, 0:ct * 8].rearrange(
                    "p (t k) -> p t k", k=8)
                st2 = stage[0:64, ct * 8:ct * 48].rearrange(
                    "p (t j) -> p t j", j=40)
                st1 = stage[64:128, 0:ct * 24].rearrange(
                    "p (t j) -> p t j", j=24)
                st3 = stage[64:128, ct * 24:ct * 80].rearrange(
                    "p (t j) -> p t j", j=56)
                for pb in range(ct // PSB):
                    p0 = pb * PSB
                    # S8[e, t, k] = (loc[e, t] == k)  (one-hot)
                    s8 = s8_pool.tile([128, PSB * 8], BF16, tag="s8")
                    nc.vector.tensor_tensor(
                        s8.rearrange("p (t k) -> p t k", t=PSB),
                        bass.AP(tensor=as_t.tensor,
                                offset=as_t.offset + p0 * 18 + 16,
                                ap=[as_t.ap[0], [18, PSB], [0, 8]]),
                        bass.AP(tensor=iota_t.tensor, offset=iota_t.offset,
                                ap=[iota_t.ap[0], [0, PSB], [1, 8]]),
                        mybir.AluOpType.is_equal,
                    )
                    at = at_pool.tile([128, PSB * 128], BF16, tag="at")
                    # At[e, t, lm*8 + k] = A[e, lm] * S8[e, k]
                    nc.vector.tensor_mul(
                        at.rearrange("p (t l k) -> p t l k", t=PSB, l=16),
                        bass.AP(tensor=as_t.tensor,
                                offset=as_t.offset + p0 * 18,
                                ap=[as_t.ap[0], [18, PSB], [1, 16],
                                    [0, 8]]),
                        bass.AP(tensor=s8.tensor, offset=s8.offset,
                                ap=[s8.ap[0], [8, PSB], [0, 16],
                                    [1, 8]]),
                    )
                    ps = ps_pool.tile([128, PSB, 128], F32, tag="ps")
                    for k in range(PSB):
                        t = p0 + k
                        nc.tensor.matmul(
                            ps[:, k, 0:32],
                            w_t[:, t * 256:t * 256 + 128],
                            at[:, k * 128:k * 128 + 32],
                            start=True, stop=True)
                        nc.tensor.matmul(
                            ps[:, k, 32:128],
                            w_t[:, t * 256 + 128:t * 256 + 256],
                            at[:, k * 128 + 32:k * 128 + 128],
                            start=True, stop=True)
                    # useful-rectangle extraction (fp32 PSUM -> bf16 stage)
                    nc.scalar.copy(st3[:, p0:p0 + PSB], ps[64:128, :, 72:128])
                    nc.scalar.copy(st2[:, p0:p0 + PSB], ps[0:64, :, 32:72])
                    nc.scalar.copy(st1[:, p0:p0 + PSB], ps[64:128, :, 8:32])
                    nc.vector.tensor_copy(st0[:, p0:p0 + PSB],
                                          ps[0:64, :, 0:8])
                # chunk-major output blocks on the Pool queue (keeps the SP
                # queue free for input prefetch): rows 0:64 = [l0 ct*8 |
                # l2 ct*40] @ t0*48, rows 64:128 = [l1 ct*24 | l3 ct*56]
                # @ t0*80
                nc.gpsimd.dma_start(
                    out=bass.AP(tensor=out_d.tensor, offset=t0 * 48,
                                ap=[[T * OB, 64], [1, ct * 48]]),
                    in_=stage[0:64, 0:ct * 48])
                nc.gpsimd.dma_start(
                    out=bass.AP(tensor=out_d.tensor,
                                offset=64 * T * OB + t0 * 80,
                                ap=[[T * OB, 64], [1, ct * 80]]),
                    in_=stage[64:128, 0:ct * 80])
    nc.compile()
    return nc


def kernel(node_feats, edge_attrs, tp_weights, receiver_list, nnodes,
           _trace=False):
    node_feats = np.asarray(node_feats)
    edge_attrs = np.asarray(edge_attrs)
    tp_weights = np.asarray(tp_weights)
    receiver_list = np.asarray(receiver_list)
    nnodes = int(nnodes)
    assert node_feats.shape == (NNODES, NCHAN) and nnodes == NNODES
    assert tp_weights.shape == (NEDGES, 4, NCHAN)

    deg, node_e0, per_core, t_u, bounds = _build_schedule(receiver_list)
    key = int(t_u)
    if key not in _PROGRAM_CACHE:
        _PROGRAM_CACHE[key] = _build_program(t_u)
    nc = _PROGRAM_CACHE[key]

    # padded-by-one edge tables (last row = zeros) for gather packing
    w_bf = np.zeros((NEDGES + 1, 256), NPBF)
    w_bf[:NEDGES] = np.asarray(tp_weights, np.float32).reshape(
        NEDGES, 256).astype(NPBF)
    a2_bf = np.zeros((NEDGES + 1, 16), NPBF)
    a2_bf[:NEDGES] = np.asarray(edge_attrs, np.float32).astype(NPBF)

    in_maps, node_maps = [], []
    for c in range(N_CORES):
        X, node_map = _pack_core(per_core[c], t_u, deg, node_e0,
                                 w_bf, a2_bf, bounds[c],
                                 bounds[c + 1] - bounds[c])
        in_maps.append({"inp": X})
        node_maps.append(node_map)

    res = run_bass_kernel_spmd(nc, in_maps, list(range(N_CORES)),
                               trace=_trace)

    T = t_u
    feats = np.asarray(node_feats, np.float32)
    out = np.empty((NNODES, 16, NCHAN), np.float32)
    for c in range(N_CORES):
        r = res.results[c]["out"].astype(np.float32)   # [128, T*80]
        lo = np.empty((64, T, 48), np.float32)
        hi = np.empty((64, T, 80), np.float32)
        for ch in range(-(-T // CHUNK)):
            t0, t1 = ch * CHUNK, min((ch + 1) * CHUNK, T)
            ct = t1 - t0
            lo_reg = r[0:64, t0 * 48:t0 * 48 + ct * 48]
            lo[:, t0:t1, 0:8] = lo_reg[:, 0:ct * 8].reshape(64, ct, 8)
            lo[:, t0:t1, 8:48] = lo_reg[:, ct * 8:].reshape(64, ct, 40)
            hi_reg = r[64:128, t0 * 80:t0 * 80 + ct * 80]
            hi[:, t0:t1, 0:24] = hi_reg[:, 0:ct * 24].reshape(64, ct, 24)
            hi[:, t0:t1, 24:80] = hi_reg[:, ct * 24:].reshape(64, ct, 56)
        b0, b1 = bounds[c], bounds[c + 1]
        n_c = b1 - b0
        S = np.empty((n_c + 1, 16, NCHAN), np.float32)
        idx = node_maps[c].ravel()                      # [T*8] local ids
        blocks = (
            (lo[:, :, 0:8].reshape(64, T, 1, 8), 0, 1),
            (hi[:, :, 0:24].reshape(64, T, 3, 8), 1, 3),
            (lo[:, :, 8:48].reshape(64, T, 5, 8), 4, 5),
            (hi[:, :, 24:80].reshape(64, T, 7, 8), 9, 7),
        )
        for blk, lm0, m in blocks:
            vals = blk.transpose(1, 3, 2, 0).reshape(T * 8, m, NCHAN)
            S[idx, lm0:lm0 + m] = vals
        out[b0:b1] = S[:n_c] * feats[b0:b1, None, :]
    if _trace:
        return out, res
    return out
